# revision 23
# baseline (speedup 1.0000x reference)
"""CenterLoss forward on 8 Trainium2 NeuronCores (Bass/Tile).

loss = mean_b ||features[b] - centers[labels[b]]||^2  (LAMBDA_C = 1.0)

Strategy — BALANCED CLASS-GROUP sharding + STREAM/GATHER split:
  - The host bin-packs classes into 8 groups so every core owns EXACTLY
    batch/8 = 8192 examples (LPT on per-class counts; the ~50k singleton
    classes make the packing exact).
  - Within a group, PRESENT classes get local indices [0, nd) and one
    REPRESENTATIVE example per distinct class is laid out in local-class
    order. Those rows' centers are exactly shard rows 0,1,2,... — a
    plain streaming DMA (full HBM rate, no descriptors, no ucode), not a
    gather. Only the remaining ~2.4k rows (duplicate-class examples +
    overflow) use the SWDGE dma_gather (random 512B reads run at only
    ~170GB/s, and each gather instruction also waits on the one-time Q7
    ucode library load ~13.6us + ~8us first-use init). This cuts the
    descriptor-gather traffic ~3.4x.
  - Rows are padded with (feature := center) so pads contribute 0.
  - Per compute chunk: DVE subtract (bf16 2x rate ~214 elem/ns), then
    square+reduce on ACT (Square + accumulator, ~131 elem/ns) for most
    chunks and a fused DVE multiply+accumulate (~120 elem/ns) for two,
    balancing the engines. Chunked input DMAs let compute start while
    streams are still in flight.
  - Data ships as bf16 (tolerance gate 2e-2; measured rel err ~1e-5).
    fp8 halves bytes but not time (gather is descriptor/512B-random
    bound) and halves DVE read rate.
  - Host sums the 8 partial scalars and divides by the batch size.
"""

import heapq

import ml_dtypes
import numpy as np

import concourse.bacc as bacc
import concourse.mybir as mybir
import concourse.tile as tile
from concourse import library_config
from concourse.bass_utils import run_bass_kernel_spmd
from concourse.dve_ops import TENSOR_TENSOR_REDUCE

NCORES = 8
BATCH = 65536
FEAT_DIM = 256
NUM_CLASSES = 100000
LAMBDA_C = 1.0
P = 128

USE_FP8 = False
USE_BF16 = True
_dt = mybir.dt.bfloat16
_np_dt = ml_dtypes.bfloat16
_cs_dt = mybir.dt.float8e4  # streamed centers: DVE has slack in the
# stream region, so the 1x-rate mixed subtract is affordable and the
# stream sheds 1.5MB off the HBM roofline
_np_cs_dt = ml_dtypes.float8_e4m3
_f32 = mybir.dt.float32
_bf16 = mybir.dt.bfloat16

NQ = 4  # SWDGE queues (ucode max)
CSHARD_MAX = 14000  # static shard row count shipped per core (>= any group)
NRB = 64  # 8192 rows per core
DISTB = 46  # stream-region blocks (5888 rows; every core has >= 6014
# distinct classes for this problem size, so the region is always full
# of real representatives)
DIST = DISTB * P
# Gather chunks (blocks) covering [DISTB, NRB): issue order -> queue c%4.
GCHUNKS = ((46, 5), (51, 5), (56, 4), (60, 4))
# Compute chunks (block ranges); squares on DVE (fused mult+accum) for two
# mid chunks, ACT for the rest — balances ACT (~2.28us/8blk incl accum
# read) against DVE (subs ~1.23us/8blk + fused squares ~2.29us/8blk).
CCHUNKS = ((0, 8), (8, 8), (16, 8), (24, 8), (32, 8), (40, 6), (46, 5), (51, 5), (56, 4), (60, 4))
DVE_SQ_CHUNKS = (9,)  # block range (60,4): DVE square, emitted after all tail subs


def _build(nrb):
    assert nrb == NRB
    nc = bacc.Bacc(
        "TRN2",
        target_bir_lowering=False,
        debug=False,
        num_devices=NCORES,
        enable_asserts=False,
        dynamic_dma_scratch_size=16384,
        num_swdge_queues=NQ,
    )
    ngather = (NRB - DISTB) * P
    feat_d = nc.dram_tensor("features", [P, nrb, FEAT_DIM], _dt, kind="ExternalInput")
    lab_d = nc.dram_tensor(
        "labels", [P, ngather // 16], mybir.dt.int16, kind="ExternalInput"
    )
    cent_d = nc.dram_tensor(
        "centers", [CSHARD_MAX, FEAT_DIM], _dt, kind="ExternalInput"
    )
    cstr_d = nc.dram_tensor(
        "cstream", [P, DISTB, FEAT_DIM], _cs_dt, kind="ExternalInput"
    )
    out_d = nc.dram_tensor("partial", [1, 1], _f32, kind="ExternalOutput")

    act_cols = [c for c in range(len(CCHUNKS)) if c not in DVE_SQ_CHUNKS]
    dve_cols = list(DVE_SQ_CHUNKS)

    with tile.TileContext(nc) as tc:
        with (
            tc.tile_pool(name="big", bufs=1) as big,
            tc.tile_pool(name="sc", bufs=2) as sc,
            tc.tile_pool(name="ps", bufs=1, space="PSUM") as ps,
        ):
            # Start the Q7 ucode IRAM load as early as possible.
            nc.gpsimd.load_library(library_config.mlp)

            # Gather indices (gather region only), wrapped [16, n/16] and
            # replicated to 128 partitions (dma_gather's expected layout).
            lab = big.tile([P, ngather // 16], mybir.dt.int16)
            nc.sync.dma_start(out=lab[:], in_=lab_d.ap())

            ones = big.tile([P, 1], _f32)
            nc.vector.memset(ones[:], 1.0)

            feat = big.tile([P, nrb, FEAT_DIM], _dt)
            cent = big.tile([P, DISTB, FEAT_DIM], _cs_dt)  # stream region
            centg = big.tile([P, nrb - DISTB, FEAT_DIM], _dt)  # gather region
            # Separate accumulators per engine: sharing one tile across ACT
            # and DVE writers created cross-engine scheduling serialization.
            accA = big.tile([P, len(act_cols)], _f32)
            accV = big.tile([P, len(dve_cols)], _f32)

            warm_idx = big.tile([P, 1], mybir.dt.int16)
            nc.vector.memset(warm_idx[:], 0)
            warm_out = big.tile([P, 1, FEAT_DIM], _dt)
            nc.gpsimd.dma_gather(
                warm_out[:], cent_d.ap(), warm_idx[:], 16, 16, FEAT_DIM
            )
            for c, (b0, cb) in enumerate(GCHUNKS):
                i0 = (b0 - DISTB) * P // 16
                g0 = b0 - DISTB
                nc.gpsimd.dma_gather(
                    centg[:, g0 : g0 + cb, :],
                    cent_d.ap(),
                    lab[:, i0 : i0 + cb * 8],
                    cb * P,
                    cb * P,
                    FEAT_DIM,
                    queue_num=((c + 1) % 8) % 4,
                )

            # Streamed center rows (locals [0, DIST)) and features, pieces
            # interleaved in issue order so compute can chase the stream.
            csb = (0, 10, 19, 28, 37, 46)
            for j in range(5):
                nc.sync.dma_start(
                    out=cent[:, csb[j] : csb[j + 1], :],
                    in_=cstr_d.ap()[:, csb[j] : csb[j + 1], :],
                )
                b0 = j * 8
                nc.sync.dma_start(
                    out=feat[:, b0 : b0 + 8, :], in_=feat_d.ap()[:, b0 : b0 + 8, :]
                )
            for b0 in range(40, nrb, 8):
                nc.sync.dma_start(
                    out=feat[:, b0 : b0 + 8, :], in_=feat_d.ap()[:, b0 : b0 + 8, :]
                )

            def emit_sub(c):
                b0, cb = CCHUNKS[c]
                if b0 < DISTB:
                    assert b0 + cb <= DISTB
                    csrc = cent[:, b0 : b0 + cb, :]
                else:
                    csrc = centg[:, b0 - DISTB : b0 - DISTB + cb, :]
                diff_t = sc.tile([P, cb, FEAT_DIM], _bf16, tag=f"diff{c}")
                nc.vector.tensor_tensor(
                    out=diff_t[:],
                    in0=feat[:, b0 : b0 + cb, :],
                    in1=csrc,
                    op=mybir.AluOpType.subtract,
                )
                return diff_t

            def emit_sq(c, diff_t):
                cb = CCHUNKS[c][1]
                if c in DVE_SQ_CHUNKS:
                    col = dve_cols.index(c)
                    sq_t = sc.tile([P, cb, FEAT_DIM], _bf16, tag=f"vsq{c}")
                    nc.vector._custom_dve(
                        TENSOR_TENSOR_REDUCE,
                        out=sq_t[:],
                        in0=diff_t[:],
                        in1=diff_t[:],
                        s0=0.0,
                        s1=1.0,
                        accum_out=accV[:, col : col + 1],
                    )
                else:
                    col = act_cols.index(c)
                    sq_t = sc.tile([P, cb, FEAT_DIM], _bf16, tag=f"asq{c}")
                    nc.scalar.activation(
                        out=sq_t[:],
                        in_=diff_t[:],
                        func=mybir.ActivationFunctionType.Square,
                        accum_out=accA[:, col : col + 1],
                    )

            for c in range(6):
                emit_sq(c, emit_sub(c))
            d6 = emit_sub(6)
            emit_sq(6, d6)
            d7 = emit_sub(7)
            emit_sq(7, d7)
            d8 = emit_sub(8)
            emit_sq(8, d8)
            d9 = emit_sub(9)
            emit_sq(9, d9)

            # accA/accV -> [128,1] -> [1,1] -> HBM
            r1 = big.tile([P, 1], _f32)
            r2 = big.tile([P, 1], _f32)
            nc.vector.reduce_sum(out=r1[:], in_=accA[:], axis=mybir.AxisListType.X)
            nc.vector.reduce_sum(out=r2[:], in_=accV[:], axis=mybir.AxisListType.X)
            acc1 = big.tile([P, 1], _f32)
            nc.vector.tensor_tensor(
                out=acc1[:], in0=r1[:], in1=r2[:], op=mybir.AluOpType.add
            )
            res_ps = ps.tile([1, 1], _f32)
            nc.tensor.matmul(
                out=res_ps[:], lhsT=acc1[:], rhs=ones[:], start=True, stop=True
            )
            res_sb = big.tile([1, 1], _f32)
            nc.vector.reduce_sum(out=res_sb[:], in_=res_ps[:], axis=mybir.AxisListType.X)
            nc.sync.dma_start(out=out_d.ap(), in_=res_sb[:])

    nc.compile()
    return nc


_nc_cache = {}


def _get_nc(nrb):
    if nrb not in _nc_cache:
        _nc_cache[nrb] = _build(nrb)
    return _nc_cache[nrb]


def _pack_classes(labels):
    """LPT bin-packing of classes into NCORES groups, balancing example
    counts. Returns (group_of_class, counts_per_core). With many singleton
    classes the packing is exact (all groups == BATCH/NCORES)."""
    counts_c = np.bincount(labels, minlength=NUM_CLASSES)
    nz = np.nonzero(counts_c)[0]
    nz = nz[np.argsort(-counts_c[nz], kind="stable")]
    group_of_class = np.empty(NUM_CLASSES, dtype=np.int8)
    heap = [(0, k) for k in range(NCORES)]
    heapq.heapify(heap)
    cc = counts_c[nz]
    for c, n in zip(nz.tolist(), cc.tolist()):
        tot, k = heapq.heappop(heap)
        group_of_class[c] = k
        heapq.heappush(heap, (tot + n, k))
    # zero-count classes: round-robin (only affects shard layout size)
    z = np.nonzero(counts_c == 0)[0]
    group_of_class[z] = np.arange(len(z)) % NCORES
    totals = np.zeros(NCORES, dtype=np.int64)
    np.add.at(totals, group_of_class[nz], counts_c[nz])
    return group_of_class, totals


def _make_in_maps(features, labels, centers):
    features = np.ascontiguousarray(np.asarray(features, dtype=np.float32))
    labels = np.ascontiguousarray(np.asarray(labels)).astype(np.int64)
    centers = np.ascontiguousarray(np.asarray(centers, dtype=np.float32))
    assert features.shape == (BATCH, FEAT_DIM)
    assert labels.shape == (BATCH,)
    assert centers.shape == (NUM_CLASSES, FEAT_DIM)

    group_of_class, counts = _pack_classes(labels)
    counts_c = np.bincount(labels, minlength=NUM_CLASSES)

    # Local class index within each group: PRESENT classes first
    # (ascending), then absent — so distinct representatives map to shard
    # rows 0..nd-1 (the streamable prefix).
    present = counts_c > 0
    keys = group_of_class.astype(np.int64) * 2 + (~present)
    order_c = np.argsort(keys, kind="stable")
    local_of_class = np.empty(NUM_CLASSES, dtype=np.int32)
    gsizes = np.bincount(group_of_class, minlength=NCORES)
    assert gsizes.max() <= CSHARD_MAX, gsizes
    starts = np.concatenate([[0], np.cumsum(gsizes)])
    for k in range(NCORES):
        cls_k = order_c[starts[k] : starts[k + 1]]
        local_of_class[cls_k] = np.arange(len(cls_k))

    bucket = group_of_class[labels]
    loc_all = local_of_class[labels]
    order = np.lexsort((loc_all, bucket))
    nr = NRB * P
    assert int(counts.max()) <= nr, counts

    cent_np = centers.astype(_np_dt)
    ngather = nr - DIST
    in_maps = []
    pos = 0
    for k in range(NCORES):
        n = int(counts[k])
        ex = order[pos : pos + n]  # this core's examples, sorted by local
        pos += n
        cls_k = order_c[starts[k] : starts[k + 1]]
        cshard = np.zeros((CSHARD_MAX, FEAT_DIM), dtype=_np_dt)
        cshard[: len(cls_k)] = cent_np[cls_k]

        loc_sorted = loc_all[ex]
        first = np.ones(n, dtype=bool)
        first[1:] = loc_sorted[1:] != loc_sorted[:-1]
        rep_pos = np.nonzero(first)[0]
        nd = len(rep_pos)
        # Streamed representatives: one example per distinct class, local
        # classes [0, min(nd, DIST)).
        nstream = min(nd, DIST)
        take = np.zeros(n, dtype=bool)
        take[rep_pos[:nstream]] = True

        feat_k = np.empty((nr, FEAT_DIM), dtype=_np_dt)
        feat_k[:nstream] = features[ex[take]].astype(_np_dt)
        # Stream pad (only if nd < DIST): feature := that center row as
        # quantized in the fp8 stream -> diff exactly 0.
        feat_k[nstream:DIST] = (
            cshard[nstream:DIST].astype(_np_cs_dt).astype(_np_dt)
        )
        # Gather region: everything else, still sorted by local class.
        rest = ex[~take]
        g = len(rest)
        assert DIST + g <= nr, (nd, g)
        feat_k[DIST : DIST + g] = features[rest].astype(_np_dt)
        feat_k[DIST + g :] = cshard[0]
        locg = np.zeros((ngather,), dtype=np.int16)
        locg[:g] = loc_all[rest].astype(np.int16)

        lab16 = np.ascontiguousarray(
            np.tile(locg.reshape(ngather // 16, 16).T, (P // 16, 1))
        )
        featw = np.ascontiguousarray(
            feat_k.reshape(NRB, P, FEAT_DIM).transpose(1, 0, 2)
        )
        cstream = np.ascontiguousarray(
            cshard[:DIST].astype(_np_cs_dt).reshape(DISTB, P, FEAT_DIM).transpose(1, 0, 2)
        )
        in_maps.append(
            {"features": featw, "labels": lab16, "centers": cshard, "cstream": cstream}
        )
    return in_maps, NRB


def _reduce_results(results):
    total = sum(float(r["partial"][0, 0]) for r in results)
    return np.float32(LAMBDA_C * total / BATCH)


def kernel(features: np.ndarray, labels: np.ndarray, centers: np.ndarray):
    in_maps, nrb = _make_in_maps(features, labels, centers)
    res = run_bass_kernel_spmd(_get_nc(nrb), in_maps, core_ids=list(range(NCORES)))
    return _reduce_results(res.results)


# revision 24
# speedup vs baseline: 1.0351x; 1.0351x over previous
"""CenterLoss forward on 8 Trainium2 NeuronCores (Bass/Tile).

loss = mean_b ||features[b] - centers[labels[b]]||^2  (LAMBDA_C = 1.0)

Strategy — BALANCED CLASS-GROUP sharding + STREAM/GATHER split:
  - The host bin-packs classes into 8 groups so every core owns EXACTLY
    batch/8 = 8192 examples (LPT on per-class counts; the ~50k singleton
    classes make the packing exact).
  - Within a group, PRESENT classes get local indices [0, nd) and one
    REPRESENTATIVE example per distinct class is laid out in local-class
    order. Those rows' centers are exactly shard rows 0,1,2,... — a
    plain streaming DMA (full HBM rate, no descriptors, no ucode), not a
    gather. Only the remaining ~2.4k rows (duplicate-class examples +
    overflow) use the SWDGE dma_gather (random 512B reads run at only
    ~170GB/s, and each gather instruction also waits on the one-time Q7
    ucode library load ~13.6us + ~8us first-use init). This cuts the
    descriptor-gather traffic ~3.4x.
  - Rows are padded with (feature := center) so pads contribute 0.
  - Per compute chunk: DVE subtract (bf16 2x rate ~214 elem/ns), then
    square+reduce on ACT (Square + accumulator, ~131 elem/ns) for most
    chunks and a fused DVE multiply+accumulate (~120 elem/ns) for two,
    balancing the engines. Chunked input DMAs let compute start while
    streams are still in flight.
  - Data ships as bf16 (tolerance gate 2e-2; measured rel err ~1e-5).
    fp8 halves bytes but not time (gather is descriptor/512B-random
    bound) and halves DVE read rate.
  - Host sums the 8 partial scalars and divides by the batch size.
"""

import heapq

import ml_dtypes
import numpy as np

import concourse.bacc as bacc
import concourse.mybir as mybir
import concourse.tile as tile
from concourse import library_config
from concourse.bass_utils import run_bass_kernel_spmd
from concourse.dve_ops import TENSOR_TENSOR_REDUCE

NCORES = 8
BATCH = 65536
FEAT_DIM = 256
NUM_CLASSES = 100000
LAMBDA_C = 1.0
P = 128

USE_FP8 = False
USE_BF16 = True
_dt = mybir.dt.bfloat16
_np_dt = ml_dtypes.bfloat16
_cs_dt = mybir.dt.float8e4  # streamed centers: DVE has slack in the
# stream region, so the 1x-rate mixed subtract is affordable and the
# stream sheds 1.5MB off the HBM roofline
_np_cs_dt = ml_dtypes.float8_e4m3
_f32 = mybir.dt.float32
_bf16 = mybir.dt.bfloat16

NQ = 4  # SWDGE queues (ucode max)
CSHARD_MAX = 14000  # static shard row count shipped per core (>= any group)
NRB = 64  # 8192 rows per core
DISTB = 46  # stream-region blocks (5888 rows; every core has >= 6014
# distinct classes for this problem size, so the region is always full
# of real representatives)
DIST = DISTB * P
# Gather chunks (blocks) covering [DISTB, NRB): issue order -> queue c%4.
GCHUNKS = ((46, 1), (47, 5), (52, 5), (57, 5), (62, 2))
# Compute chunks (block ranges); squares on DVE (fused mult+accum) for two
# mid chunks, ACT for the rest — balances ACT (~2.28us/8blk incl accum
# read) against DVE (subs ~1.23us/8blk + fused squares ~2.29us/8blk).
CCHUNKS = ((0, 8), (8, 8), (16, 8), (24, 8), (32, 8), (40, 6), (46, 1), (47, 5), (52, 5), (57, 5), (62, 2))
DVE_SQ_CHUNKS = (9,)  # block range (57,5): DVE square, emitted after all tail subs


def _build(nrb):
    assert nrb == NRB
    nc = bacc.Bacc(
        "TRN2",
        target_bir_lowering=False,
        debug=False,
        num_devices=NCORES,
        enable_asserts=False,
        dynamic_dma_scratch_size=16384,
        num_swdge_queues=NQ,
    )
    ngather = (NRB - DISTB) * P
    feat_d = nc.dram_tensor("features", [P, nrb, FEAT_DIM], _dt, kind="ExternalInput")
    lab_d = nc.dram_tensor(
        "labels", [P, ngather // 16], mybir.dt.int16, kind="ExternalInput"
    )
    cent_d = nc.dram_tensor(
        "centers", [CSHARD_MAX, FEAT_DIM], _dt, kind="ExternalInput"
    )
    cstr_d = nc.dram_tensor(
        "cstream", [P, DISTB, FEAT_DIM], _cs_dt, kind="ExternalInput"
    )
    out_d = nc.dram_tensor("partial", [1, 1], _f32, kind="ExternalOutput")

    act_cols = [c for c in range(len(CCHUNKS)) if c not in DVE_SQ_CHUNKS]
    dve_cols = list(DVE_SQ_CHUNKS)

    with tile.TileContext(nc) as tc:
        with (
            tc.tile_pool(name="big", bufs=1) as big,
            tc.tile_pool(name="sc", bufs=2) as sc,
            tc.tile_pool(name="ps", bufs=1, space="PSUM") as ps,
        ):
            # Start the Q7 ucode IRAM load as early as possible.
            nc.gpsimd.load_library(library_config.mlp)

            # Gather indices (gather region only), wrapped [16, n/16] and
            # replicated to 128 partitions (dma_gather's expected layout).
            lab = big.tile([P, ngather // 16], mybir.dt.int16)
            nc.sync.dma_start(out=lab[:], in_=lab_d.ap())

            ones = big.tile([P, 1], _f32)
            nc.vector.memset(ones[:], 1.0)

            feat = big.tile([P, nrb, FEAT_DIM], _dt)
            cent = big.tile([P, DISTB, FEAT_DIM], _cs_dt)  # stream region
            centg = big.tile([P, nrb - DISTB, FEAT_DIM], _dt)  # gather region
            # Separate accumulators per engine: sharing one tile across ACT
            # and DVE writers created cross-engine scheduling serialization.
            accA = big.tile([P, len(act_cols)], _f32)
            accV = big.tile([P, len(dve_cols)], _f32)

            for c, (b0, cb) in enumerate(GCHUNKS):
                i0 = (b0 - DISTB) * P // 16
                g0 = b0 - DISTB
                nc.gpsimd.dma_gather(
                    centg[:, g0 : g0 + cb, :],
                    cent_d.ap(),
                    lab[:, i0 : i0 + cb * 8],
                    cb * P,
                    cb * P,
                    FEAT_DIM,
                    queue_num=(c % 8) % 4,
                )

            # Streamed center rows (locals [0, DIST)) and features, pieces
            # interleaved in issue order so compute can chase the stream.
            csb = (0, 10, 19, 28, 37, 46)
            for j in range(5):
                nc.sync.dma_start(
                    out=cent[:, csb[j] : csb[j + 1], :],
                    in_=cstr_d.ap()[:, csb[j] : csb[j + 1], :],
                )
                b0 = j * 8
                nc.sync.dma_start(
                    out=feat[:, b0 : b0 + 8, :], in_=feat_d.ap()[:, b0 : b0 + 8, :]
                )
            for b0 in range(40, nrb, 8):
                nc.sync.dma_start(
                    out=feat[:, b0 : b0 + 8, :], in_=feat_d.ap()[:, b0 : b0 + 8, :]
                )

            def emit_sub(c):
                b0, cb = CCHUNKS[c]
                if b0 < DISTB:
                    assert b0 + cb <= DISTB
                    csrc = cent[:, b0 : b0 + cb, :]
                else:
                    csrc = centg[:, b0 - DISTB : b0 - DISTB + cb, :]
                diff_t = sc.tile([P, cb, FEAT_DIM], _bf16, tag=f"diff{c}")
                nc.vector.tensor_tensor(
                    out=diff_t[:],
                    in0=feat[:, b0 : b0 + cb, :],
                    in1=csrc,
                    op=mybir.AluOpType.subtract,
                )
                return diff_t

            def emit_sq(c, diff_t):
                cb = CCHUNKS[c][1]
                if c in DVE_SQ_CHUNKS:
                    col = dve_cols.index(c)
                    sq_t = sc.tile([P, cb, FEAT_DIM], _bf16, tag=f"vsq{c}")
                    nc.vector._custom_dve(
                        TENSOR_TENSOR_REDUCE,
                        out=sq_t[:],
                        in0=diff_t[:],
                        in1=diff_t[:],
                        s0=0.0,
                        s1=1.0,
                        accum_out=accV[:, col : col + 1],
                    )
                else:
                    col = act_cols.index(c)
                    sq_t = sc.tile([P, cb, FEAT_DIM], _bf16, tag=f"asq{c}")
                    nc.scalar.activation(
                        out=sq_t[:],
                        in_=diff_t[:],
                        func=mybir.ActivationFunctionType.Square,
                        accum_out=accA[:, col : col + 1],
                    )

            for c in range(7):
                emit_sq(c, emit_sub(c))
            d7 = emit_sub(7)
            emit_sq(7, d7)
            d8 = emit_sub(8)
            emit_sq(8, d8)
            d9 = emit_sub(9)
            d10 = emit_sub(10)
            emit_sq(10, d10)
            emit_sq(9, d9)

            # accA/accV -> [128,1] -> [1,1] -> HBM
            r1 = big.tile([P, 1], _f32)
            r2 = big.tile([P, 1], _f32)
            nc.vector.reduce_sum(out=r1[:], in_=accA[:], axis=mybir.AxisListType.X)
            nc.vector.reduce_sum(out=r2[:], in_=accV[:], axis=mybir.AxisListType.X)
            acc1 = big.tile([P, 1], _f32)
            nc.vector.tensor_tensor(
                out=acc1[:], in0=r1[:], in1=r2[:], op=mybir.AluOpType.add
            )
            res_ps = ps.tile([1, 1], _f32)
            nc.tensor.matmul(
                out=res_ps[:], lhsT=acc1[:], rhs=ones[:], start=True, stop=True
            )
            res_sb = big.tile([1, 1], _f32)
            nc.vector.reduce_sum(out=res_sb[:], in_=res_ps[:], axis=mybir.AxisListType.X)
            nc.sync.dma_start(out=out_d.ap(), in_=res_sb[:])

    nc.compile()
    return nc


_nc_cache = {}


def _get_nc(nrb):
    if nrb not in _nc_cache:
        _nc_cache[nrb] = _build(nrb)
    return _nc_cache[nrb]


def _pack_classes(labels):
    """LPT bin-packing of classes into NCORES groups, balancing example
    counts. Returns (group_of_class, counts_per_core). With many singleton
    classes the packing is exact (all groups == BATCH/NCORES)."""
    counts_c = np.bincount(labels, minlength=NUM_CLASSES)
    nz = np.nonzero(counts_c)[0]
    nz = nz[np.argsort(-counts_c[nz], kind="stable")]
    group_of_class = np.empty(NUM_CLASSES, dtype=np.int8)
    heap = [(0, k) for k in range(NCORES)]
    heapq.heapify(heap)
    cc = counts_c[nz]
    for c, n in zip(nz.tolist(), cc.tolist()):
        tot, k = heapq.heappop(heap)
        group_of_class[c] = k
        heapq.heappush(heap, (tot + n, k))
    # zero-count classes: round-robin (only affects shard layout size)
    z = np.nonzero(counts_c == 0)[0]
    group_of_class[z] = np.arange(len(z)) % NCORES
    totals = np.zeros(NCORES, dtype=np.int64)
    np.add.at(totals, group_of_class[nz], counts_c[nz])
    return group_of_class, totals


def _make_in_maps(features, labels, centers):
    features = np.ascontiguousarray(np.asarray(features, dtype=np.float32))
    labels = np.ascontiguousarray(np.asarray(labels)).astype(np.int64)
    centers = np.ascontiguousarray(np.asarray(centers, dtype=np.float32))
    assert features.shape == (BATCH, FEAT_DIM)
    assert labels.shape == (BATCH,)
    assert centers.shape == (NUM_CLASSES, FEAT_DIM)

    group_of_class, counts = _pack_classes(labels)
    counts_c = np.bincount(labels, minlength=NUM_CLASSES)

    # Local class index within each group: PRESENT classes first
    # (ascending), then absent — so distinct representatives map to shard
    # rows 0..nd-1 (the streamable prefix).
    present = counts_c > 0
    keys = group_of_class.astype(np.int64) * 2 + (~present)
    order_c = np.argsort(keys, kind="stable")
    local_of_class = np.empty(NUM_CLASSES, dtype=np.int32)
    gsizes = np.bincount(group_of_class, minlength=NCORES)
    assert gsizes.max() <= CSHARD_MAX, gsizes
    starts = np.concatenate([[0], np.cumsum(gsizes)])
    for k in range(NCORES):
        cls_k = order_c[starts[k] : starts[k + 1]]
        local_of_class[cls_k] = np.arange(len(cls_k))

    bucket = group_of_class[labels]
    loc_all = local_of_class[labels]
    order = np.lexsort((loc_all, bucket))
    nr = NRB * P
    assert int(counts.max()) <= nr, counts

    cent_np = centers.astype(_np_dt)
    ngather = nr - DIST
    in_maps = []
    pos = 0
    for k in range(NCORES):
        n = int(counts[k])
        ex = order[pos : pos + n]  # this core's examples, sorted by local
        pos += n
        cls_k = order_c[starts[k] : starts[k + 1]]
        cshard = np.zeros((CSHARD_MAX, FEAT_DIM), dtype=_np_dt)
        cshard[: len(cls_k)] = cent_np[cls_k]

        loc_sorted = loc_all[ex]
        first = np.ones(n, dtype=bool)
        first[1:] = loc_sorted[1:] != loc_sorted[:-1]
        rep_pos = np.nonzero(first)[0]
        nd = len(rep_pos)
        # Streamed representatives: one example per distinct class, local
        # classes [0, min(nd, DIST)).
        nstream = min(nd, DIST)
        take = np.zeros(n, dtype=bool)
        take[rep_pos[:nstream]] = True

        feat_k = np.empty((nr, FEAT_DIM), dtype=_np_dt)
        feat_k[:nstream] = features[ex[take]].astype(_np_dt)
        # Stream pad (only if nd < DIST): feature := that center row as
        # quantized in the fp8 stream -> diff exactly 0.
        feat_k[nstream:DIST] = (
            cshard[nstream:DIST].astype(_np_cs_dt).astype(_np_dt)
        )
        # Gather region: everything else, still sorted by local class.
        rest = ex[~take]
        g = len(rest)
        assert DIST + g <= nr, (nd, g)
        feat_k[DIST : DIST + g] = features[rest].astype(_np_dt)
        feat_k[DIST + g :] = cshard[0]
        locg = np.zeros((ngather,), dtype=np.int16)
        locg[:g] = loc_all[rest].astype(np.int16)

        lab16 = np.ascontiguousarray(
            np.tile(locg.reshape(ngather // 16, 16).T, (P // 16, 1))
        )
        featw = np.ascontiguousarray(
            feat_k.reshape(NRB, P, FEAT_DIM).transpose(1, 0, 2)
        )
        cstream = np.ascontiguousarray(
            cshard[:DIST].astype(_np_cs_dt).reshape(DISTB, P, FEAT_DIM).transpose(1, 0, 2)
        )
        in_maps.append(
            {"features": featw, "labels": lab16, "centers": cshard, "cstream": cstream}
        )
    return in_maps, NRB


def _reduce_results(results):
    total = sum(float(r["partial"][0, 0]) for r in results)
    return np.float32(LAMBDA_C * total / BATCH)


def kernel(features: np.ndarray, labels: np.ndarray, centers: np.ndarray):
    in_maps, nrb = _make_in_maps(features, labels, centers)
    res = run_bass_kernel_spmd(_get_nc(nrb), in_maps, core_ids=list(range(NCORES)))
    return _reduce_results(res.results)


# revision 25
# speedup vs baseline: 1.0378x; 1.0026x over previous
"""CenterLoss forward on 8 Trainium2 NeuronCores (Bass/Tile).

loss = mean_b ||features[b] - centers[labels[b]]||^2  (LAMBDA_C = 1.0)

Strategy — BALANCED CLASS-GROUP sharding + STREAM/GATHER split:
  - The host bin-packs classes into 8 groups so every core owns EXACTLY
    batch/8 = 8192 examples (LPT on per-class counts; the ~50k singleton
    classes make the packing exact).
  - Within a group, PRESENT classes get local indices [0, nd) and one
    REPRESENTATIVE example per distinct class is laid out in local-class
    order. Those rows' centers are exactly shard rows 0,1,2,... — a
    plain streaming DMA (full HBM rate, no descriptors, no ucode), not a
    gather. Only the remaining ~2.4k rows (duplicate-class examples +
    overflow) use the SWDGE dma_gather (random 512B reads run at only
    ~170GB/s, and each gather instruction also waits on the one-time Q7
    ucode library load ~13.6us + ~8us first-use init). This cuts the
    descriptor-gather traffic ~3.4x.
  - Rows are padded with (feature := center) so pads contribute 0.
  - Per compute chunk: DVE subtract (bf16 2x rate ~214 elem/ns), then
    square+reduce on ACT (Square + accumulator, ~131 elem/ns) for most
    chunks and a fused DVE multiply+accumulate (~120 elem/ns) for two,
    balancing the engines. Chunked input DMAs let compute start while
    streams are still in flight.
  - Data ships as bf16 (tolerance gate 2e-2; measured rel err ~1e-5).
    fp8 halves bytes but not time (gather is descriptor/512B-random
    bound) and halves DVE read rate.
  - Host sums the 8 partial scalars and divides by the batch size.
"""

import heapq

import ml_dtypes
import numpy as np

import concourse.bacc as bacc
import concourse.mybir as mybir
import concourse.tile as tile
from concourse import library_config
from concourse.bass_utils import run_bass_kernel_spmd
from concourse.dve_ops import TENSOR_TENSOR_REDUCE

NCORES = 8
BATCH = 65536
FEAT_DIM = 256
NUM_CLASSES = 100000
LAMBDA_C = 1.0
P = 128

USE_FP8 = False
USE_BF16 = True
_dt = mybir.dt.bfloat16
_np_dt = ml_dtypes.bfloat16
_cs_dt = mybir.dt.float8e4  # streamed centers: DVE has slack in the
# stream region, so the 1x-rate mixed subtract is affordable and the
# stream sheds 1.5MB off the HBM roofline
_np_cs_dt = ml_dtypes.float8_e4m3
_f32 = mybir.dt.float32
_bf16 = mybir.dt.bfloat16

NQ = 4  # SWDGE queues (ucode max)
CSHARD_MAX = 14000  # static shard row count shipped per core (>= any group)
NRB = 64  # 8192 rows per core
DISTB = 46  # stream-region blocks (5888 rows; every core has >= 6014
# distinct classes for this problem size, so the region is always full
# of real representatives)
DIST = DISTB * P
# Gather chunks (blocks) covering [DISTB, NRB): issue order -> queue c%4.
GCHUNKS = ((46, 1), (47, 5), (52, 5), (57, 5), (62, 2))
# Compute chunks (block ranges); squares on DVE (fused mult+accum) for two
# mid chunks, ACT for the rest — balances ACT (~2.28us/8blk incl accum
# read) against DVE (subs ~1.23us/8blk + fused squares ~2.29us/8blk).
CCHUNKS = ((0, 8), (8, 8), (16, 8), (24, 8), (32, 8), (40, 6), (46, 1), (47, 5), (52, 5), (57, 5), (62, 2))
DVE_SQ_CHUNKS = (8,)  # block range (52,5): DVE square, emitted after all tail subs


def _build(nrb):
    assert nrb == NRB
    nc = bacc.Bacc(
        "TRN2",
        target_bir_lowering=False,
        debug=False,
        num_devices=NCORES,
        enable_asserts=False,
        dynamic_dma_scratch_size=16384,
        num_swdge_queues=NQ,
    )
    ngather = (NRB - DISTB) * P
    feat_d = nc.dram_tensor("features", [P, nrb, FEAT_DIM], _dt, kind="ExternalInput")
    lab_d = nc.dram_tensor(
        "labels", [P, ngather // 16], mybir.dt.int16, kind="ExternalInput"
    )
    cent_d = nc.dram_tensor(
        "centers", [CSHARD_MAX, FEAT_DIM], _dt, kind="ExternalInput"
    )
    cstr_d = nc.dram_tensor(
        "cstream", [P, DISTB, FEAT_DIM], _cs_dt, kind="ExternalInput"
    )
    out_d = nc.dram_tensor("partial", [1, 1], _f32, kind="ExternalOutput")

    act_cols = [c for c in range(len(CCHUNKS)) if c not in DVE_SQ_CHUNKS]
    dve_cols = list(DVE_SQ_CHUNKS)

    with tile.TileContext(nc) as tc:
        with (
            tc.tile_pool(name="big", bufs=1) as big,
            tc.tile_pool(name="sc", bufs=2) as sc,
            tc.tile_pool(name="ps", bufs=1, space="PSUM") as ps,
        ):
            # Start the Q7 ucode IRAM load as early as possible.
            nc.gpsimd.load_library(library_config.mlp)

            # Gather indices (gather region only), wrapped [16, n/16] and
            # replicated to 128 partitions (dma_gather's expected layout).
            lab = big.tile([P, ngather // 16], mybir.dt.int16)
            nc.sync.dma_start(out=lab[:], in_=lab_d.ap())

            ones = big.tile([P, 1], _f32)
            nc.vector.memset(ones[:], 1.0)

            feat = big.tile([P, nrb, FEAT_DIM], _dt)
            cent = big.tile([P, DISTB, FEAT_DIM], _cs_dt)  # stream region
            centg = big.tile([P, nrb - DISTB, FEAT_DIM], _dt)  # gather region
            # Separate accumulators per engine: sharing one tile across ACT
            # and DVE writers created cross-engine scheduling serialization.
            accA = big.tile([P, len(act_cols)], _f32)
            accV = big.tile([P, len(dve_cols)], _f32)

            for c, (b0, cb) in enumerate(GCHUNKS):
                i0 = (b0 - DISTB) * P // 16
                g0 = b0 - DISTB
                nc.gpsimd.dma_gather(
                    centg[:, g0 : g0 + cb, :],
                    cent_d.ap(),
                    lab[:, i0 : i0 + cb * 8],
                    cb * P,
                    cb * P,
                    FEAT_DIM,
                    queue_num=(c % 8) % 4,
                )

            # Streamed center rows (locals [0, DIST)) and features, pieces
            # interleaved in issue order so compute can chase the stream.
            csb = (0, 10, 19, 28, 37, 46)
            for j in range(5):
                nc.sync.dma_start(
                    out=cent[:, csb[j] : csb[j + 1], :],
                    in_=cstr_d.ap()[:, csb[j] : csb[j + 1], :],
                )
                b0 = j * 8
                nc.sync.dma_start(
                    out=feat[:, b0 : b0 + 8, :], in_=feat_d.ap()[:, b0 : b0 + 8, :]
                )
            for b0 in range(40, nrb, 8):
                nc.sync.dma_start(
                    out=feat[:, b0 : b0 + 8, :], in_=feat_d.ap()[:, b0 : b0 + 8, :]
                )

            def emit_sub(c):
                b0, cb = CCHUNKS[c]
                if b0 < DISTB:
                    assert b0 + cb <= DISTB
                    csrc = cent[:, b0 : b0 + cb, :]
                else:
                    csrc = centg[:, b0 - DISTB : b0 - DISTB + cb, :]
                diff_t = sc.tile([P, cb, FEAT_DIM], _bf16, tag=f"diff{c}")
                nc.vector.tensor_tensor(
                    out=diff_t[:],
                    in0=feat[:, b0 : b0 + cb, :],
                    in1=csrc,
                    op=mybir.AluOpType.subtract,
                )
                return diff_t

            def emit_sq(c, diff_t):
                cb = CCHUNKS[c][1]
                if c in DVE_SQ_CHUNKS:
                    col = dve_cols.index(c)
                    sq_t = sc.tile([P, cb, FEAT_DIM], _bf16, tag=f"vsq{c}")
                    nc.vector._custom_dve(
                        TENSOR_TENSOR_REDUCE,
                        out=sq_t[:],
                        in0=diff_t[:],
                        in1=diff_t[:],
                        s0=0.0,
                        s1=1.0,
                        accum_out=accV[:, col : col + 1],
                    )
                else:
                    col = act_cols.index(c)
                    sq_t = sc.tile([P, cb, FEAT_DIM], _bf16, tag=f"asq{c}")
                    nc.scalar.activation(
                        out=sq_t[:],
                        in_=diff_t[:],
                        func=mybir.ActivationFunctionType.Square,
                        accum_out=accA[:, col : col + 1],
                    )

            for c in range(7):
                emit_sq(c, emit_sub(c))
            d7 = emit_sub(7)
            emit_sq(7, d7)
            d8 = emit_sub(8)
            d9 = emit_sub(9)
            emit_sq(9, d9)
            d10 = emit_sub(10)
            emit_sq(10, d10)
            emit_sq(8, d8)

            # accA/accV -> [128,1] -> [1,1] -> HBM
            r1 = big.tile([P, 1], _f32)
            r2 = big.tile([P, 1], _f32)
            nc.vector.reduce_sum(out=r1[:], in_=accA[:], axis=mybir.AxisListType.X)
            nc.vector.reduce_sum(out=r2[:], in_=accV[:], axis=mybir.AxisListType.X)
            acc1 = big.tile([P, 1], _f32)
            nc.vector.tensor_tensor(
                out=acc1[:], in0=r1[:], in1=r2[:], op=mybir.AluOpType.add
            )
            res_ps = ps.tile([1, 1], _f32)
            nc.tensor.matmul(
                out=res_ps[:], lhsT=acc1[:], rhs=ones[:], start=True, stop=True
            )
            res_sb = big.tile([1, 1], _f32)
            nc.vector.reduce_sum(out=res_sb[:], in_=res_ps[:], axis=mybir.AxisListType.X)
            nc.sync.dma_start(out=out_d.ap(), in_=res_sb[:])

    nc.compile()
    return nc


_nc_cache = {}


def _get_nc(nrb):
    if nrb not in _nc_cache:
        _nc_cache[nrb] = _build(nrb)
    return _nc_cache[nrb]


def _pack_classes(labels):
    """LPT bin-packing of classes into NCORES groups, balancing example
    counts. Returns (group_of_class, counts_per_core). With many singleton
    classes the packing is exact (all groups == BATCH/NCORES)."""
    counts_c = np.bincount(labels, minlength=NUM_CLASSES)
    nz = np.nonzero(counts_c)[0]
    nz = nz[np.argsort(-counts_c[nz], kind="stable")]
    group_of_class = np.empty(NUM_CLASSES, dtype=np.int8)
    heap = [(0, k) for k in range(NCORES)]
    heapq.heapify(heap)
    cc = counts_c[nz]
    for c, n in zip(nz.tolist(), cc.tolist()):
        tot, k = heapq.heappop(heap)
        group_of_class[c] = k
        heapq.heappush(heap, (tot + n, k))
    # zero-count classes: round-robin (only affects shard layout size)
    z = np.nonzero(counts_c == 0)[0]
    group_of_class[z] = np.arange(len(z)) % NCORES
    totals = np.zeros(NCORES, dtype=np.int64)
    np.add.at(totals, group_of_class[nz], counts_c[nz])
    return group_of_class, totals


def _make_in_maps(features, labels, centers):
    features = np.ascontiguousarray(np.asarray(features, dtype=np.float32))
    labels = np.ascontiguousarray(np.asarray(labels)).astype(np.int64)
    centers = np.ascontiguousarray(np.asarray(centers, dtype=np.float32))
    assert features.shape == (BATCH, FEAT_DIM)
    assert labels.shape == (BATCH,)
    assert centers.shape == (NUM_CLASSES, FEAT_DIM)

    group_of_class, counts = _pack_classes(labels)
    counts_c = np.bincount(labels, minlength=NUM_CLASSES)

    # Local class index within each group: PRESENT classes first
    # (ascending), then absent — so distinct representatives map to shard
    # rows 0..nd-1 (the streamable prefix).
    present = counts_c > 0
    keys = group_of_class.astype(np.int64) * 2 + (~present)
    order_c = np.argsort(keys, kind="stable")
    local_of_class = np.empty(NUM_CLASSES, dtype=np.int32)
    gsizes = np.bincount(group_of_class, minlength=NCORES)
    assert gsizes.max() <= CSHARD_MAX, gsizes
    starts = np.concatenate([[0], np.cumsum(gsizes)])
    for k in range(NCORES):
        cls_k = order_c[starts[k] : starts[k + 1]]
        local_of_class[cls_k] = np.arange(len(cls_k))

    bucket = group_of_class[labels]
    loc_all = local_of_class[labels]
    order = np.lexsort((loc_all, bucket))
    nr = NRB * P
    assert int(counts.max()) <= nr, counts

    cent_np = centers.astype(_np_dt)
    ngather = nr - DIST
    in_maps = []
    pos = 0
    for k in range(NCORES):
        n = int(counts[k])
        ex = order[pos : pos + n]  # this core's examples, sorted by local
        pos += n
        cls_k = order_c[starts[k] : starts[k + 1]]
        cshard = np.zeros((CSHARD_MAX, FEAT_DIM), dtype=_np_dt)
        cshard[: len(cls_k)] = cent_np[cls_k]

        loc_sorted = loc_all[ex]
        first = np.ones(n, dtype=bool)
        first[1:] = loc_sorted[1:] != loc_sorted[:-1]
        rep_pos = np.nonzero(first)[0]
        nd = len(rep_pos)
        # Streamed representatives: one example per distinct class, local
        # classes [0, min(nd, DIST)).
        nstream = min(nd, DIST)
        take = np.zeros(n, dtype=bool)
        take[rep_pos[:nstream]] = True

        feat_k = np.empty((nr, FEAT_DIM), dtype=_np_dt)
        feat_k[:nstream] = features[ex[take]].astype(_np_dt)
        # Stream pad (only if nd < DIST): feature := that center row as
        # quantized in the fp8 stream -> diff exactly 0.
        feat_k[nstream:DIST] = (
            cshard[nstream:DIST].astype(_np_cs_dt).astype(_np_dt)
        )
        # Gather region: everything else, still sorted by local class.
        rest = ex[~take]
        g = len(rest)
        assert DIST + g <= nr, (nd, g)
        feat_k[DIST : DIST + g] = features[rest].astype(_np_dt)
        feat_k[DIST + g :] = cshard[0]
        locg = np.zeros((ngather,), dtype=np.int16)
        locg[:g] = loc_all[rest].astype(np.int16)

        lab16 = np.ascontiguousarray(
            np.tile(locg.reshape(ngather // 16, 16).T, (P // 16, 1))
        )
        featw = np.ascontiguousarray(
            feat_k.reshape(NRB, P, FEAT_DIM).transpose(1, 0, 2)
        )
        cstream = np.ascontiguousarray(
            cshard[:DIST].astype(_np_cs_dt).reshape(DISTB, P, FEAT_DIM).transpose(1, 0, 2)
        )
        in_maps.append(
            {"features": featw, "labels": lab16, "centers": cshard, "cstream": cstream}
        )
    return in_maps, NRB


def _reduce_results(results):
    total = sum(float(r["partial"][0, 0]) for r in results)
    return np.float32(LAMBDA_C * total / BATCH)


def kernel(features: np.ndarray, labels: np.ndarray, centers: np.ndarray):
    in_maps, nrb = _make_in_maps(features, labels, centers)
    res = run_bass_kernel_spmd(_get_nc(nrb), in_maps, core_ids=list(range(NCORES)))
    return _reduce_results(res.results)
